# revision 1
# baseline (speedup 1.0000x reference)
"""Trainium2 Bass kernel for nn_BlocksCore (RIMs BlocksCore fwd step), v2.

Contract: kernel(**inputs) takes FULL unsharded inputs (np arrays, keyed as in
setup_inputs) and returns the FULL output tuple (hx_out [8192,1024] f32,
mask_full [8192,1024] f32), matching reference().

Strategy: pure data-parallel over batch (1024 samples/core on 8 cores).
Feature-major device layout ([features, batch]); host pre-fuses weights
(Wv1[1] @ gru_wi) and does the final mask/combine (hx + 0.5*mask*delta2).

v2 design:
- single ACT table set: every sigmoid is rewritten via tanh identities
  (sigmoid(u) = (1+tanh(u/2))/2), exp+tanh live in exp_and_others.
- GRU input-side matmuls in fp8 DoubleRow (contraction 256 in one MM).
- partition-replication (att weights, exp, recip) via SBUF->SBUF DMA
  row-repeat instead of PE selector matmuls.
- score/mask path kept in f32 for exact top-k ranking.
- device emits delta2 = 2*(zes+att) bf16; host: hx + 0.5*mask*delta2.
"""

import numpy as np
import ml_dtypes
from contextlib import ExitStack

import concourse.bass as bass
import concourse.bacc as bacc
import concourse.tile as tile
import concourse.mybir as mybir
from concourse.bass_utils import run_bass_kernel_spmd

AF = mybir.ActivationFunctionType
OP = mybir.AluOpType
f32 = mybir.dt.float32
bf16 = mybir.dt.bfloat16
fp8 = mybir.dt.float8e4
BF = ml_dtypes.bfloat16
F8 = ml_dtypes.float8_e4m3fn

B, NINP, NHID = 8192, 256, 1024
NCORES = 8
BC = B // NCORES          # 1024 per core
F = 512                   # batch-tile columns
NT = BC // F              # 2 tiles
NB = 8                    # output blocks
BS = 128                  # block size
S8 = 32.0                 # fp8 weight scale


def _build_consts():
    """Constant selector matrices."""
    c = {}
    # s1 partition-sum: prod[p] [128=(a2,e64), F] -> s1 [8, F]; col 2p+a
    m = np.zeros((4, 128, 8), np.float32)
    for p in range(4):
        m[p, 0:64, 2 * p] = 1
        m[p, 64:128, 2 * p + 1] = 1
    c["c_s1sum"] = m.transpose(1, 0, 2).reshape(128, 32)  # slice [:, p*8:(p+1)*8]

    # mask diff: diff[8i+j] = s1[j] - s1[i]
    pq = np.zeros((8, 64), np.float32)
    for i in range(8):
        for j in range(8):
            pq[j, 8 * i + j] += 1
            pq[i, 8 * i + j] -= 1
    c["pq"] = pq

    # rank: rank[i] = sum_j g[8i+j]  (bf16: 0/1 exact)
    r64 = np.zeros((64, 8), np.float32)
    for i in range(8):
        for j in range(8):
            r64[8 * i + j, i] = 1
    c["r64"] = r64

    # comm-attn QK sum: prod2(i,r) rows (a,h,d)=64a+16h+d -> s_i row 4(2r+a)+h
    m = np.zeros((4, 128, 32), np.float32)
    for r in range(4):
        for a in range(2):
            for h in range(4):
                for d in range(16):
                    m[r, 64 * a + 16 * h + d, 4 * (2 * r + a) + h] = 1
    c["c_qksum"] = m.transpose(1, 0, 2).reshape(128, 128)  # [:, r*32:(r+1)*32]

    # denom: expS_i row 4j+h -> denom row 4i+h (sum over j)
    m = np.zeros((8, 32, 32), np.float32)
    for i in range(8):
        for j in range(8):
            for h in range(4):
                m[i, 4 * j + h, 4 * i + h] = 1
    c["c_den"] = m.transpose(1, 0, 2).reshape(32, 8 * 32)  # [:, i*32:(i+1)*32]

    # fold: avp rows 64a+16h+d -> o rows 16h+d (sum over a)
    fold = np.zeros((128, 64), np.float32)
    for a in range(2):
        for h in range(4):
            for d in range(16):
                fold[64 * a + 16 * h + d, 16 * h + d] = 1
    c["fold"] = fold

    # DR-packed qksum: [p, (half:2, j:2, m:32)]
    qk8 = np.zeros((128, 128), np.float32)
    for half in range(2):
        for j in range(2):
            qk8[:, half * 64 + j * 32: half * 64 + j * 32 + 32] = \
                c["c_qksum"][:, (2 * half + j) * 32: (2 * half + j) * 32 + 32]
    c["c_qk8"] = qk8
    # DR-packed fold: [p, (a:2, j:2, m:128)]; a=0 -> out rows 0:64, a=1 -> 64:128
    f8c = np.zeros((128, 512), np.float32)
    for a_ in range(2):
        for j in range(2):
            f8c[:, a_ * 256 + j * 128 + 64 * a_: a_ * 256 + j * 128 + 64 * a_ + 64] = fold
    c["c_fold8"] = f8c
    return c


_CONSTS = _build_consts()
_PROGRAM = None


def _build_program():
    nc = bacc.Bacc("TRN2", target_bir_lowering=False, debug=False)

    def din(name, shape, dt=bf16):
        return nc.dram_tensor(name, shape, dt, kind="ExternalInput")

    # per-core activations
    inpTf = din("inpTf", [NINP, BC], f32)
    inpTh = din("inpTh", [NINP, BC])          # inp/2 bf16
    hxT = din("hxT", [NHID, BC], f32)
    hxTh = din("hxTh", [NHID, BC])            # hx/2 bf16
    # weights (shared)
    wq1 = din("wq1", [128, 512], f32)
    wk1 = din("wk1", [128, 128], f32)
    wfu8 = din("wfu8", [128, NB * 3 * 256], fp8)   # per (k,gate): [2,128] DR-packed
    wh = din("wh", [128, 3072])                    # r,z: 2S*wh ; n: S*wh
    wq2d = din("wq2d", [128, NB * 128])            # 2*Wq2 dup'd cols
    wk2 = din("wk2", [128, 512])                   # 2*Wk2
    wv2 = din("wv2", [128, 512])                   # 2*Wv2
    fcg = din("fcg", [128, 256])                   # [fc_w; fc_w | gate_w; gate_w]
    # biases f32 [128, n]
    b_rt = din("b_rt", [128, 8], f32)         # 0.5*(gbi_r+gbh_r)
    b_zt = din("b_zt", [128, 8], f32)         # 0.5*(gbi_z+gbh_z)
    b_rhn = din("b_rhn", [128, 8], f32)       # S*0.5*gbh_n
    b_n = din("b_n", [128, 8], f32)           # gbi_n + 0.5*gbh_n
    b_fg = din("b_fg", [128, 2], f32)         # fc_b ; 0.5*gate_b
    # selector consts: f32 for the score path, bf16 for phase C
    cs_f32 = ("c_s1sum", "pq", "r64")
    cs_b16 = ("c_qksum", "c_den", "fold")
    cs_fp8 = ()
    cs_names = cs_f32 + cs_b16
    csd = {n: din("c_" + n, list(_CONSTS[n].shape),
                  f32 if n in cs_f32 else bf16) for n in cs_names}

    a2T = nc.dram_tensor("a2T", [NHID, BC], bf16, kind="ExternalOutput")
    wT = nc.dram_tensor("wT", [NHID, BC], bf16, kind="ExternalOutput")
    mask8 = nc.dram_tensor("mask8", [8, BC], f32, kind="ExternalOutput")

    with ExitStack() as ctx:
        tc = ctx.enter_context(tile.TileContext(nc))
        wp = ctx.enter_context(tc.tile_pool(name="wp", bufs=1))       # weights
        sb = ctx.enter_context(tc.tile_pool(name="sb", bufs=1))       # per-tile
        ak = ctx.enter_context(tc.tile_pool(name="ak", bufs=3))       # 1KB transients
        ab = ctx.enter_context(tc.tile_pool(name="ab", bufs=3))       # 4KB transients
        ap4 = ctx.enter_context(tc.tile_pool(name="ap4", bufs=3))     # f32 prods
        ps = ctx.enter_context(tc.tile_pool(name="ps", bufs=5, space="PSUM"))
        ps2 = ctx.enter_context(tc.tile_pool(name="ps2", bufs=3, space="PSUM"))

        # ---- per-tile input loads for tile 0 come FIRST (before big weights)
        inpf_t = [[None, None] for _ in range(NT)]
        hx_t = [[None] * 8 for _ in range(NT)]
        inph_t = [None] * NT
        hxh_t = [[None] * 8 for _ in range(NT)]

        def load_tile_inputs(t):
            # inpf/hx are phase-A-only: shared tags (single buffer across tiles)
            # spread loads over 3 queues to parallelize trigger issue + DGE
            sl = bass.ts(t, F)
            for cch in range(2):
                inpf_t[t][cch] = sb.tile([128, F], f32, tag=f"inpf{cch}",
                                         name=f"inpf{cch}")
                nc.sync.dma_start(inpf_t[t][cch][:],
                                   inpTf.ap()[bass.ts(cch, 128), sl])
            for k in range(8):
                hx_t[t][k] = sb.tile([128, F], f32, tag=f"hx{k}", name=f"hx{k}")
                nc.sync.dma_start(hx_t[t][k][:], hxT.ap()[bass.ts(k, 128), sl])
            inph_t[t] = sb.tile([128, 2 * F], bf16, tag="inph", name="inph")
            for cch in range(2):
                nc.sync.dma_start(inph_t[t][:, bass.ts(cch, F)],
                                  inpTh.ap()[bass.ts(cch, 128), sl])
            for k in range(8):
                hxh_t[t][k] = sb.tile([128, F], bf16, tag=f"hxh{k}",
                                      name=f"hxh{k}")
                nc.sync.dma_start(hxh_t[t][k][:], hxTh.ap()[bass.ts(k, 128), sl])

        def wtile(dram, shape, dt=bf16):
            t = wp.tile(shape, dt, tag=dram.name, name=dram.name)
            nc.sync.dma_start(t[:], dram.ap())
            return t

        # small score-path weights first, then tile-0 inputs, then the rest
        W = {}
        W["wq1"] = wtile(wq1, [128, 512], f32)
        W["wk1"] = wtile(wk1, [128, 128], f32)
        C = {n: wtile(csd[n], list(_CONSTS[n].shape), f32) for n in cs_f32}
        load_tile_inputs(0)
        W["wfu8"] = wtile(wfu8, [128, NB * 3 * 256], fp8)
        W["wh"] = wtile(wh, [128, 3072])
        W["wq2d"] = wtile(wq2d, [128, NB * 128])
        W["wk2"] = wtile(wk2, [128, 512])
        W["wv2"] = wtile(wv2, [128, 512])
        W["fcg"] = wtile(fcg, [128, 256])
        for d, shp in [(b_rt, [128, 8]), (b_zt, [128, 8]), (b_rhn, [128, 8]),
                       (b_n, [128, 8]), (b_fg, [128, 2])]:
            W[d.name] = wtile(d, shp, f32)
        for n in cs_b16:
            C[n] = wtile(csd[n], list(_CONSTS[n].shape), bf16)

        for t in range(NT):
            sl = bass.ts(t, F)
            if t > 0:
                load_tile_inputs(t)
            inpf, hx, inph, hxh = inpf_t[t], hx_t[t], inph_t[t], hxh_t[t]

            # ---- phase A: input attention scores + mask (f32 exact) ----
            kk_ps = ps.tile([128, F], f32, tag="ps128", name="ps128")
            for cch in range(2):
                nc.tensor.matmul(kk_ps[0:64, :], W["wk1"][:, bass.ts(cch, 64)],
                                 inpf[cch][:], start=(cch == 0), stop=(cch == 1))
            for cch in range(2):
                nc.tensor.matmul(kk_ps[64:128, :], W["wk1"][:, bass.ts(cch, 64)],
                                 inpf[cch][:], start=(cch == 0), stop=(cch == 1),
                                 tile_position=(0, 64))
            kkS = sb.tile([128, F], f32, tag="kkS", name="kkS")
            nc.scalar.copy(kkS[:], kk_ps[:])

            prods = []
            for p in range(4):
                q_ps = ps.tile([128, F], f32, tag="ps128", name="ps128")
                nc.tensor.matmul(q_ps[0:64, :], W["wq1"][:, bass.ts(2 * p, 64)],
                                 hx[2 * p][:], start=True, stop=True)
                nc.tensor.matmul(q_ps[64:128, :], W["wq1"][:, bass.ts(2 * p + 1, 64)],
                                 hx[2 * p + 1][:], start=True, stop=True,
                                 tile_position=(0, 64))
                pr = ap4.tile([128, F], f32, tag="prod", name="prod")
                nc.vector.tensor_tensor(pr[:], q_ps[:], kkS[:], OP.mult)
                prods.append(pr)

            s1_ps = ps2.tile([8, F], f32, tag="psS", name="psS")
            for p in range(4):
                nc.tensor.matmul(s1_ps[:], C["c_s1sum"][:, bass.ts(p, 8)], prods[p][:],
                                 start=(p == 0), stop=(p == 3))
            s1S = sb.tile([8, F], f32, tag="s1S", name="s1S")
            nc.scalar.copy(s1S[:], s1_ps[:])
            # att weight in tanh form: t_att = tanh(s1/16); att = (1+t_att)/2
            tat8 = sb.tile([8, F], bf16, tag="tat8", name="tat8")
            nc.scalar.activation(tat8[:], s1_ps[:], AF.Tanh, scale=0.0625)

            diff_ps = ps2.tile([64, F], f32, tag="psS", name="psS")
            nc.tensor.matmul(diff_ps[:], C["pq"][:], s1S[:], start=True, stop=True)
            g = sb.tile([64, F], bf16, tag="g", name="g")
            nc.vector.tensor_single_scalar(g[:], diff_ps[:], 0.0, OP.is_gt)
            r64b = sb.tile([64, 8], bf16, tag="r64b", name="r64b")
            nc.scalar.copy(r64b[:], C["r64"][:])
            rank_ps = ps2.tile([8, F], f32, tag="psS", name="psS")
            nc.tensor.matmul(rank_ps[:], r64b[:], g[:], start=True, stop=True)
            m8 = sb.tile([8, F], bf16, tag="m8", name="m8")
            nc.vector.tensor_single_scalar(m8[:], rank_ps[:], 3.5, OP.is_le)
            nc.gpsimd.dma_start(mask8.ap()[:, sl], m8[:])

            # replicate t_att rows via DMA row-bcast (SWDGE only — HWDGE
            # silently mis-handles stride-0 sources)
            attT = [None] * 8
            for k in range(8):
                at = ak.tile([128, F], bf16, tag="attT", name="attT")
                nc.gpsimd.dma_start(at[:], tat8[k:k + 1, :].unsqueeze(1)
                                    .to_broadcast([1, 128, F]))
                attT[k] = at[:]

            # ---- phase B: block GRU (fp8 DoubleRow on input side) ----
            w_t = [None] * 8      # w = (t_z-1)*(n-hx) = -2*zes, bf16
            hprh = [None] * 8     # hpr/2 bf16
            for k in range(8):
                xk8 = ak.tile([128, 2 * F], fp8, tag="xk8", name="xk8")
                for cch in range(2):
                    nc.vector.scalar_tensor_tensor(
                        xk8[:, bass.ts(cch, F)], attT[k], 1.0,
                        inph[:, bass.ts(cch, F)], OP.add, OP.mult)
                xk8v = xk8[:].rearrange("p (c b) -> p c b", c=2)
                kb8 = k * 768
                kbh = k * 384
                gate_ps = {}
                for gi, gn in enumerate(("r", "z", "n")):
                    gp = ps.tile([128, F], f32, tag="ps128", name="ps128")
                    nc.tensor.matmul(
                        gp[:],
                        W["wfu8"][:, kb8 + gi * 256: kb8 + gi * 256 + 256]
                        .rearrange("p (j m) -> p j m", j=2),
                        xk8v, start=True, stop=False,
                        perf_mode=mybir.MatmulPerfMode.DoubleRow)
                    nc.tensor.matmul(gp[:],
                                     W["wh"][:, kbh + gi * 128: kbh + gi * 128 + 128],
                                     hxh[k][:], start=False, stop=True)
                    gate_ps[gn] = gp
                hn_ps = ps.tile([128, F], f32, tag="ps128", name="ps128")
                nc.tensor.matmul(hn_ps[:], W["wh"][:, kbh + 256: kbh + 384],
                                 hxh[k][:], start=True, stop=True)

                t_r = ak.tile([128, F], bf16, tag="t_r", name="t_r")
                nc.scalar.activation(t_r[:], gate_ps["r"][:], AF.Tanh,
                                     scale=0.5 / S8, bias=W["b_rt"][:, k: k + 1])
                t_z = ak.tile([128, F], bf16, tag="t_z", name="t_z")
                nc.scalar.activation(t_z[:], gate_ps["z"][:], AF.Tanh,
                                     scale=0.5 / S8, bias=W["b_zt"][:, k: k + 1])
                rhn_t = ak.tile([128, F], bf16, tag="rhn_t", name="rhn_t")
                nc.vector.scalar_tensor_tensor(rhn_t[:], hn_ps[:],
                                               W["b_rhn"][:, k: k + 1], t_r[:],
                                               OP.add, OP.mult)
                npre_cp = ak.tile([128, F], bf16, tag="npre_cp", name="npre_cp")
                nc.scalar.copy(npre_cp[:], gate_ps["n"][:])
                npre2 = ak.tile([128, F], bf16, tag="npre2", name="npre2")
                nc.vector.tensor_tensor(npre2[:], npre_cp[:], rhn_t[:], OP.add)
                n = ak.tile([128, F], bf16, tag="n", name="n")
                nc.scalar.activation(n[:], npre2[:], AF.Tanh,
                                     scale=1.0 / S8, bias=W["b_n"][:, k: k + 1])
                e2 = ak.tile([128, F], bf16, tag="e2", name="e2")
                nc.vector.scalar_tensor_tensor(e2[:], hxh[k][:], -2.0, n[:],
                                               OP.mult, OP.add)
                w_t[k] = sb.tile([128, F], bf16, tag=f"w{k}", name=f"w{k}")
                nc.vector.scalar_tensor_tensor(w_t[k][:], t_z[:], -1.0, e2[:],
                                               OP.add, OP.mult)
                hprh[k] = sb.tile([128, F], bf16, tag=f"hprh{k}", name=f"hprh{k}")
                nc.vector.scalar_tensor_tensor(hprh[k][:], w_t[k][:], -0.25,
                                               hxh[k][:], OP.mult, OP.add)

            # ---- phase C: communication attention ----
            k2all = sb.tile([128, 4 * F], bf16, tag="k2all", name="k2all")
            v2all = sb.tile([128, 4 * F], bf16, tag="v2all", name="v2all")
            for rr in range(4):
                kp = ps.tile([128, F], f32, tag="ps128", name="ps128")
                nc.tensor.matmul(kp[0:64, :], W["wk2"][:, bass.ts(2 * rr, 64)],
                                 hprh[2 * rr][:], start=True, stop=True)
                nc.tensor.matmul(kp[64:128, :], W["wk2"][:, bass.ts(2 * rr + 1, 64)],
                                 hprh[2 * rr + 1][:], start=True, stop=True,
                                 tile_position=(0, 64))
                nc.scalar.copy(k2all[:, bass.ts(rr, F)], kp[:])
                vp = ps.tile([128, F], f32, tag="ps128", name="ps128")
                nc.tensor.matmul(vp[0:64, :], W["wv2"][:, bass.ts(2 * rr, 64)],
                                 hprh[2 * rr][:], start=True, stop=True)
                nc.tensor.matmul(vp[64:128, :], W["wv2"][:, bass.ts(2 * rr + 1, 64)],
                                 hprh[2 * rr + 1][:], start=True, stop=True,
                                 tile_position=(0, 64))
                nc.scalar.copy(v2all[:, bass.ts(rr, F)], vp[:])

            expS = [None] * 8
            for i in range(8):
                qp = ps.tile([128, F], f32, tag="ps128", name="ps128")
                nc.tensor.matmul(qp[:], W["wq2d"][:, bass.ts(i, 128)], hprh[i][:],
                                 start=True, stop=True)
                qdS = ak.tile([128, F], bf16, tag="qdS", name="qdS")
                nc.scalar.copy(qdS[:], qp[:])
                pr2 = ab.tile([128, 4 * F], bf16, tag="pr2", name="pr2")
                s_ps = ps2.tile([32, F], f32, tag="psS", name="psS")
                for rr in range(4):
                    nc.vector.tensor_tensor(pr2[:, bass.ts(rr, F)], qdS[:],
                                            k2all[:, bass.ts(rr, F)], OP.mult)
                    nc.tensor.matmul(s_ps[:], C["c_qksum"][:, bass.ts(rr, 32)],
                                     pr2[:, bass.ts(rr, F)],
                                     start=(rr == 0), stop=(rr == 3))
                expS[i] = sb.tile([32, F], bf16, tag=f"expS{t}{i}", name=f"expS{i}")
                nc.scalar.activation(expS[i][:], s_ps[:], AF.Exp, scale=0.25)

            den_ps = ps2.tile([32, F], f32, tag="psS", name="psS")
            for i in range(8):
                nc.tensor.matmul(den_ps[:], C["c_den"][:, bass.ts(i, 32)], expS[i][:],
                                 start=(i == 0), stop=(i == 7))
            recipF = sb.tile([32, F], f32, tag="recipF", name="recipF")
            with nc.allow_low_precision(reason="softmax denom >=1, approx recip ok"):
                nc.vector.reciprocal_approx_fast(recipF[:], den_ps[:])
            recipS = sb.tile([32, F], bf16, tag="recipS", name="recipS")
            nc.scalar.copy(recipS[:], recipF[:])

            oS = [None] * 4
            for cc in range(4):
                on_ps = ps.tile([128, F], f32, tag="ps128", name="ps128")
                for a in range(2):
                    i = 2 * cc + a
                    # erep: dst row p <- expS_i row p//16 (8 rows repeat x16)
                    erep = ab.tile([128, 4 * F], bf16, tag="erep", name="erep")
                    avp = ab.tile([128, 4 * F], bf16, tag="avp", name="avp")
                    for rr in range(4):
                        nc.gpsimd.dma_start(
                            erep[:, bass.ts(rr, F)],
                            expS[i][8 * rr: 8 * rr + 8, :]
                            .unsqueeze(1).to_broadcast([8, 16, F]))
                        nc.vector.tensor_tensor(avp[:, bass.ts(rr, F)],
                                                erep[:, bass.ts(rr, F)],
                                                v2all[:, bass.ts(rr, F)], OP.mult)
                    for rr in range(4):
                        if a == 0:
                            nc.tensor.matmul(on_ps[0:64, :], C["fold"][:],
                                             avp[:, bass.ts(rr, F)],
                                             start=(rr == 0), stop=(rr == 3))
                        else:
                            nc.tensor.matmul(on_ps[64:128, :], C["fold"][:],
                                             avp[:, bass.ts(rr, F)],
                                             start=(rr == 0), stop=(rr == 3),
                                             tile_position=(0, 64))
                # rrep: dst row p <- recipS row 4(2cc+a)+h = row (8cc + p//16)
                rrepB = ak.tile([128, F], bf16, tag="rrepB", name="rrepB")
                nc.gpsimd.dma_start(
                    rrepB[:],
                    recipS[8 * cc: 8 * cc + 8, :].unsqueeze(1)
                    .to_broadcast([8, 16, F]))
                oc = ak.tile([128, F], bf16, tag="oc", name="oc")
                nc.scalar.copy(oc[:], on_ps[:])
                oS[cc] = sb.tile([128, F], bf16, tag=f"oS{cc}", name=f"oS{cc}")
                nc.vector.tensor_tensor(oS[cc][:], oc[:], rrepB[:], OP.mult)

            # fc / gate (row-packed pairs) + delta2 output
            for cc in range(4):
                fg_ps = [None, None]
                for a in range(2):
                    osrc = oS[cc][bass.ts(a, 64), :]
                    wsl = W["fcg"][bass.ts(a, 64), :]
                    fc_ps = ps.tile([128, F], f32, tag="ps128", name="ps128")
                    nc.tensor.matmul(fc_ps[:], wsl[:, 0:128], osrc, start=True,
                                     stop=True, tile_position=(64 * a, 0))
                    gt_ps = ps.tile([128, F], f32, tag="ps128", name="ps128")
                    nc.tensor.matmul(gt_ps[:], wsl[:, 128:256], osrc, start=True,
                                     stop=True, tile_position=(64 * a, 0))
                    fg_ps[a] = (fc_ps, gt_ps)
                for a in range(2):
                    k = 2 * cc + a
                    fc_ps, gt_ps = fg_ps[a]
                    th = ak.tile([128, F], bf16, tag="th", name="th")
                    nc.scalar.activation(th[:], fc_ps[:], AF.Tanh,
                                         bias=W["b_fg"][:, 0:1])
                    t_g = ak.tile([128, F], bf16, tag="t_g", name="t_g")
                    nc.scalar.activation(t_g[:], gt_ps[:], AF.Tanh, scale=0.5,
                                         bias=W["b_fg"][:, 1:2])
                    att2 = ak.tile([128, F], bf16, tag="att2", name="att2")
                    nc.vector.scalar_tensor_tensor(att2[:], t_g[:], 1.0, th[:],
                                                   OP.add, OP.mult)
                    nc.sync.dma_start(a2T.ap()[bass.ts(k, 128), sl], att2[:])
                    nc.gpsimd.dma_start(wT.ap()[bass.ts(k, 128), sl], w_t[k][:])

    nc.compile()
    return nc


def _prep_shared(inputs):
    """Host-side weight prep (shared across cores)."""
    g = lambda k: np.asarray(inputs[k], np.float32)
    Wq1, Wk1, Wv1 = g("Wq1"), g("Wk1"), g("Wv1")
    Wq2, Wk2, Wv2 = g("Wq2"), g("Wk2"), g("Wv2")
    fc_w, fc_b, gate_w, gate_b = g("fc_w"), g("fc_b"), g("gate_w"), g("gate_b")
    gwi, gwh, gbi, gbh = g("gru_wi"), g("gru_wh"), g("gru_bi"), g("gru_bh")

    sh = {}
    sh["wq1"] = np.ascontiguousarray(Wq1.transpose(1, 0, 2).reshape(128, 512))
    sh["wk1"] = np.ascontiguousarray(
        Wk1[1].reshape(2, 128, 64).transpose(1, 0, 2).reshape(128, 128))

    # wfu = Wv1[1] @ gru_wi : [8, 256, 384]; fp8 DR pack [p, (k, gate, j, m)]
    wf = np.einsum("de,kef->kdf", Wv1[1], gwi) * S8
    w8 = np.zeros((128, NB * 3 * 256), np.float32)
    for k in range(8):
        for gi in range(3):
            for j in range(2):
                blk = wf[k, 128 * j:128 * j + 128, 128 * gi:128 * gi + 128]
                w8[:, k * 768 + gi * 256 + j * 128: k * 768 + gi * 256 + j * 128 + 128] = blk
    sh["wfu8"] = w8.astype(F8)

    # wh: r,z gates *2S (consumed with hx/2); n gate *S (gives S*0.5*hn)
    whs = gwh.copy()
    whs[:, :, 0:256] *= 2 * S8
    whs[:, :, 256:384] *= S8
    sh["wh"] = np.ascontiguousarray(whs.transpose(1, 0, 2).reshape(128, 3072)).astype(BF)

    # q2 weights doubled (hpr/2) and column-duplicated into both halves
    wq2a = np.zeros((128, NB * 128), np.float32)
    for i in range(8):
        wq2a[:, i * 128: i * 128 + 64] = 2 * Wq2.transpose(1, 0, 2)[:, i, :]
        wq2a[:, i * 128 + 64: i * 128 + 128] = 2 * Wq2.transpose(1, 0, 2)[:, i, :]
    sh["wq2d"] = wq2a.astype(BF)
    sh["wk2"] = np.ascontiguousarray(
        (2 * Wk2).transpose(1, 0, 2).reshape(128, 512)).astype(BF)
    sh["wv2"] = np.ascontiguousarray(
        (2 * Wv2).transpose(1, 0, 2).reshape(128, 512)).astype(BF)

    fg = np.zeros((128, 256), np.float32)
    fg[0:64, 0:128] = fc_w
    fg[64:128, 0:128] = fc_w
    fg[0:64, 128:256] = gate_w
    fg[64:128, 128:256] = gate_w
    sh["fcg"] = fg.astype(BF)

    b_rt = np.zeros((128, 8), np.float32)
    b_zt = np.zeros((128, 8), np.float32)
    b_rhn = np.zeros((128, 8), np.float32)
    b_n = np.zeros((128, 8), np.float32)
    for k in range(8):
        b_rt[:, k] = 0.5 * (gbi[k, 0:128] + gbh[k, 0:128])
        b_zt[:, k] = 0.5 * (gbi[k, 128:256] + gbh[k, 128:256])
        b_rhn[:, k] = S8 * 0.5 * gbh[k, 256:384]
        b_n[:, k] = gbi[k, 256:384] + 0.5 * gbh[k, 256:384]
    sh["b_rt"], sh["b_zt"], sh["b_rhn"], sh["b_n"] = b_rt, b_zt, b_rhn, b_n
    bfg = np.zeros((128, 2), np.float32)
    bfg[:, 0] = fc_b
    bfg[:, 1] = 0.5 * gate_b
    sh["b_fg"] = bfg
    for k, v in _CONSTS.items():
        if k in ("c_qk8", "c_fold8"):
            continue
        if k in ("c_qksum", "c_den", "fold"):
            sh["c_" + k] = v.astype(BF)
        else:
            sh["c_" + k] = v.astype(np.float32)
    return sh


def _core_inputs(sh, inp, hx, c):
    s = slice(c * BC, (c + 1) * BC)
    m = dict(sh)
    inpTc = np.ascontiguousarray(inp[s].T)
    m["inpTf"] = inpTc
    m["inpTh"] = (inpTc * 0.5).astype(BF)
    hxTc = np.ascontiguousarray(hx[s].T)
    m["hxT"] = hxTc
    m["hxTh"] = (hxTc * 0.5).astype(BF)
    return m


def kernel(**inputs):
    global _PROGRAM
    if _PROGRAM is None:
        _PROGRAM = _build_program()
    nc = _PROGRAM

    inp = np.asarray(inputs["inp"], np.float32)
    hx = np.asarray(inputs["hx"], np.float32)

    sh = _prep_shared(inputs)
    in_maps = [_core_inputs(sh, inp, hx, c) for c in range(NCORES)]

    res = run_bass_kernel_spmd(nc, in_maps, list(range(NCORES)))
    hx_out = np.empty((B, NHID), np.float32)
    mask_full = np.empty((B, NHID), np.float32)
    for c in range(NCORES):
        s = slice(c * BC, (c + 1) * BC)
        mf = np.repeat(res.results[c]["mask8"].T, 128, axis=1)
        mask_full[s] = mf
        a2 = np.asarray(res.results[c]["a2T"].T, np.float32)
        w = np.asarray(res.results[c]["wT"].T, np.float32)
        hx_out[s] = hx[s] + 0.5 * mf * (a2 - w)
    return hx_out, mask_full

